# revision 23
# baseline (speedup 1.0000x reference)
"""MHA + RoPE fused kernel for Trainium2, sharded tensor-parallel over heads
across 8 NeuronCores.

Problem (hardcoded): B=4, S=2048, E=1024, H=16 heads, D=64.
  xq = x @ wq.T ; xk = x @ wk.T ; xv = x @ wv.T          [B,S,H,D]
  RoPE(xq, xk) with angles dt[b,s] * inv_freq[r]
  scores = softmax(xq @ xk.T / sqrt(D) + mask)            per (b, head)
  out = (scores @ xv) reshaped to [B,S,E]; y = out @ wo.T + bo

Sharding: each core owns 2 heads (128 channels of q/k/v) and the matching
128 rows of wo.T; it computes a full partial y (row-parallel output
projection) and the host sums the 8 partials (the "all-reduce" on host).

v3: phase 2 runs per (batch, 512-query chunk) with a 16-beat j-loop:
  score-MM pair (2 heads row-packed) -> one [128,1024] exp (both heads)
  -> 2 AV matmuls accumulating into [65,512] pos banks (ones-column carries
  the softmax denominator).  Everything else (QKV projections, RoPE, the
  softmax normalization of the previous chunk, output projection, input
  DMAs) is emitted as fine-grained filler thunks pulled inside the beat
  loop, so the PE never idles (HAM stays at full clock) while ScalarE
  streams the exps.  The softmax 1/colsum row is broadcast across
  partitions with a K=1 TensorE matmul (ones[1,64].T @ recip[1,512]).
PSUM: scores 2x[128,1024] (4 banks) + pos 2x[65,512] (2) + fillers (2) = 8.
"""

import sys

sys.path.insert(0, "/opt/trn_rl_repo")

from collections import deque

import numpy as np
import ml_dtypes

import concourse.bass as bass
from concourse import bacc
import concourse.tile as tile
from concourse import mybir
from concourse.bass_utils import run_bass_kernel_spmd

F32 = mybir.dt.float32
F16 = mybir.dt.float16
BF16 = mybir.dt.bfloat16

B, S, E, H, D = 4, 2048, 1024, 16, 64
T = B * S                      # 8192 flattened tokens
NCORES = 8
HPC = H // NCORES              # 2 heads per core
CPC = HPC * D                  # 128 channels per core
NCHUNK = T // 512              # 16 token chunks
KT = E // 128                  # 8 contraction tiles
THETA = 10000.0
NEG_INF = -1e30

_prog_cache = {}


def _build_program(use_mask: bool):
    """One Bass program, identical on every core (data differs per core)."""
    nc = bacc.Bacc()

    xT_d = nc.dram_tensor("xT", [E, T], BF16, kind="ExternalInput")
    cc_d = nc.dram_tensor("cc", [128, T], F16, kind="ExternalInput")
    ss_d = nc.dram_tensor("ss", [128, T], F16, kind="ExternalInput")
    wq_d = nc.dram_tensor("wqT", [E, CPC], BF16, kind="ExternalInput")
    wk_d = nc.dram_tensor("wkT", [E, CPC], BF16, kind="ExternalInput")
    wv_d = nc.dram_tensor("wvT", [E, CPC], BF16, kind="ExternalInput")
    wo_d = nc.dram_tensor("woT", [CPC, E], BF16, kind="ExternalInput")
    scr_d = nc.dram_tensor("csscr", [B * 4, 1024], F32)  # colsum bounce rows
    mb_d = None
    if use_mask:
        mb_d = nc.dram_tensor("mb", [128, B * 16], F32, kind="ExternalInput")
    y_d = nc.dram_tensor("yT", [E, T], F16, kind="ExternalOutput")

    xT_r = xT_d.rearrange("(k p) t -> p k t", p=128)
    wq_r = wq_d.rearrange("(k p) c -> p k c", p=128)
    wk_r = wk_d.rearrange("(k p) c -> p k c", p=128)
    wv_r = wv_d.rearrange("(k p) c -> p k c", p=128)
    wo_r = wo_d.rearrange("p (k c) -> p k c", c=128)

    with tile.TileContext(nc) as tc:
        with (
            tc.tile_pool(name="consts", bufs=1) as consts,
            tc.tile_pool(name="big", bufs=1) as big,
            tc.tile_pool(name="ph1", bufs=6) as ph1,
            tc.tile_pool(name="rope", bufs=4) as rope_pool,
            tc.tile_pool(name="pt", bufs=6) as ptp,
            tc.tile_pool(name="norm", bufs=4) as norm,
            tc.tile_pool(name="ph3", bufs=3) as ph3,
            tc.tile_pool(name="psS", bufs=2, space="PSUM") as psS,
            tc.tile_pool(name="psP", bufs=2, space="PSUM") as psP,
            tc.tile_pool(name="psF", bufs=2, space="PSUM") as psF,
        ):
            # ---- constants ----
            wq_sb = consts.tile([128, KT, CPC], BF16)
            wk_sb = consts.tile([128, KT, CPC], BF16)
            wv_sb = consts.tile([128, KT, CPC], BF16)
            wo_sb = consts.tile([128, KT, 128], BF16)
            ones_sb = consts.tile([1, 64], F32)
            nc.vector.memset(ones_sb, 1.0)
            nc.sync.dma_start(wq_sb, wq_r)
            nc.sync.dma_start(wk_sb, wk_r)
            nc.sync.dma_start(wv_sb, wv_r)
            nc.sync.dma_start(wo_sb, wo_r)
            mb_sb = None
            if use_mask:
                mb_sb = consts.tile([128, B * 16], F32)
                nc.sync.dma_start(mb_sb, mb_d[:, :])

            # ---- persistent activations ----
            qT_sb = big.tile([128, NCHUNK, 512], BF16)
            kT_sb = big.tile([128, NCHUNK, 512], BF16)
            vA_sb = big.tile([128, T // 128, 65], BF16)
            vB_sb = big.tile([128, T // 128, 65], BF16)
            attnT_sb = big.tile([128, NCHUNK, 512], BF16)
            nc.vector.memset(vA_sb[:, :, 64], 1.0)
            nc.vector.memset(vB_sb[:, :, 64], 1.0)

            def phase1_load(ch):
                """Issue the DMAs for token chunk ch; returns the tiles."""
                xsb = ph1.tile([128, KT, 512], BF16, tag="xsb")
                for k0 in (0, 4):   # two DMAs -> parallel queues
                    nc.sync.dma_start(
                        xsb[:, k0:k0 + 4, :],
                        xT_r[:, k0:k0 + 4, ch * 512:(ch + 1) * 512])
                cc_sb = ph1.tile([128, 512], F16, tag="cc")
                ss_sb = ph1.tile([128, 512], F16, tag="ss")
                nc.sync.dma_start(cc_sb, cc_d[:, ch * 512:(ch + 1) * 512])
                nc.sync.dma_start(ss_sb, ss_d[:, ch * 512:(ch + 1) * 512])
                return xsb, cc_sb, ss_sb

            def gen_qk(ch, st, w_sb, dstT):
                """Micro-steps of one q-or-k projection + RoPE for chunk ch.

                Yields (pe_cost_ns, thunk) pairs."""
                me = {}

                def mm(k0):
                    if k0 == 0:
                        me["ps"] = psF.tile([128, 512], F32, tag="fill",
                                            name="ps_qk")
                    for k in range(k0, k0 + 4):
                        nc.tensor.matmul(me["ps"], w_sb[:, k, :],
                                         st["t"][0][:, k, :],
                                         start=(k == 0), stop=(k == KT - 1))

                def rope_a():
                    # right after the last accumulation matmul: the two
                    # trig products free the psF slot as soon as they run
                    ps = me["ps"]
                    _, cc_sb, ss_sb = st["t"]
                    me["t1"] = rope_pool.tile([128, 512], BF16, tag="t1",
                                              name="t1")
                    me["t2"] = rope_pool.tile([128, 512], BF16, tag="t2",
                                              name="t2")
                    me["t2sw"] = rope_pool.tile([128, 512], BF16, tag="t2sw",
                                                name="t2sw")
                    nc.vector.tensor_tensor(me["t1"], ps, cc_sb,
                                            mybir.AluOpType.mult)
                    nc.vector.tensor_tensor(me["t2"], ps, ss_sb,
                                            mybir.AluOpType.mult)
                    for b0 in (0, 64):
                        nc.gpsimd.dma_start(me["t2sw"][b0:b0 + 32],
                                            me["t2"][b0 + 32:b0 + 64])
                        nc.gpsimd.dma_start(me["t2sw"][b0 + 32:b0 + 64],
                                            me["t2"][b0:b0 + 32])

                def rope_b():
                    nc.vector.tensor_tensor(dstT[:, ch, :], me["t1"],
                                            me["t2sw"], mybir.AluOpType.add)

                yield (870, lambda: mm(0))
                yield (900, lambda: (mm(4), rope_a()))
                yield (100, rope_b)

            def gen_v(ch, st):
                """Micro-steps of the v projection for chunk ch."""
                def mm(tt):
                    psv = psF.tile([128, 128], F32, tag="fill", name="psv")
                    for k in range(KT):
                        nc.tensor.matmul(
                            psv, st["t"][0][:, k, tt * 128:(tt + 1) * 128],
                            wv_sb[:, k, :],
                            start=(k == 0), stop=(k == KT - 1))
                    ti = ch * 4 + tt
                    nc.vector.tensor_copy(vA_sb[:, ti, 0:64], psv[:, 0:64])
                    nc.vector.tensor_copy(vB_sb[:, ti, 0:64], psv[:, 64:128])

                for tt in range(4):
                    yield (660, lambda tt=tt: mm(tt))

            def gen_load(ch, st):
                def load():
                    st["t"] = phase1_load(ch)

                yield (0, load)

            def gen_out(b, cck):
                """Micro-steps of output projection rows cck for batch b."""
                me = {}

                def step(tc4):
                    if tc4 == 0:
                        me["ysb"] = ph3.tile([128, 4, 512], F16, tag="ysb",
                                             name="ysb")
                    ch = b * 4 + tc4
                    psy = psF.tile([128, 512], F32, tag="fill", name="psy")
                    nc.tensor.matmul(psy, wo_sb[:, cck, :],
                                     attnT_sb[:, ch, :],
                                     start=True, stop=True)
                    nc.vector.tensor_copy(me["ysb"][:, tc4, :], psy)
                    if tc4 == 3:
                        nc.sync.dma_start(
                            y_d[cck * 128:(cck + 1) * 128,
                                b * 2048:(b + 1) * 2048],
                            me["ysb"])

                for tc4 in range(4):
                    yield (450, lambda tc4=tc4: step(tc4))

            def gen_finalize(b, qc, pos):
                """Normalize the attention output of (b, qc): 4 thunks.

                Holds at most one psF slot at a time (csrep) so interleaved
                filler units keep a free slot; frees both pos banks by the
                end of the second thunk."""
                ch_i = b * 4 + qc
                me = {}

                def s1():
                    me["rec"] = []
                    for hh in range(2):
                        csrow = norm.tile([1, 512], F32, tag="csrow")
                        nc.vector.tensor_copy(csrow, pos[hh][64:65, :])
                        cs_rec = norm.tile([1, 512], F32, tag="csrec")
                        nc.vector.reciprocal_approx_fast(out=cs_rec, in_=csrow)
                        me["rec"].append(cs_rec)

                def s2():
                    me["ocp"] = []
                    for hh in range(2):
                        ocp = norm.tile([64, 512], F32, tag="ocp")
                        nc.vector.tensor_copy(ocp, pos[hh][0:64, :])
                        me["ocp"].append(ocp)
                    me["rep0"] = psF.tile([64, 512], F32, tag="fill",
                                          name="csrep")
                    nc.tensor.matmul(me["rep0"], ones_sb, me["rec"][0],
                                     start=True, stop=True)

                def s3():
                    nc.vector.tensor_tensor(
                        attnT_sb[0:64, ch_i, :],
                        me["ocp"][0], me["rep0"], mybir.AluOpType.mult)
                    me["rep1"] = psF.tile([64, 512], F32, tag="fill",
                                          name="csrep")
                    nc.tensor.matmul(me["rep1"], ones_sb, me["rec"][1],
                                     start=True, stop=True)

                def s4():
                    nc.vector.tensor_tensor(
                        attnT_sb[64:128, ch_i, :],
                        me["ocp"][1], me["rep1"], mybir.AluOpType.mult)

                yield from (s1, s2, s3, s4)

            def phase2_qc(b, qc, fillers, pre, budget=700, must=None):
                """Attention for batch b, query chunk qc (512 queries).

                One-beat software pipeline: the score matmuls for beat i
                are emitted a beat ahead of exp(i), and the AV matmuls lag
                exp by a beat, so the exps run back-to-back on ScalarE
                while the PE digests score+AV+filler work.

                pre: finalize thunks of the previous chunk, popped 2/beat.
                fillers: (pe_cost, thunk) queue; each beat pulls the head
                thunk unconditionally, then more while they fit budget."""
                ch_i = b * 4 + qc
                pos = None
                pss_h = {}
                pT_h = {}
                for i in range(17):
                    # pinned thunks must be emitted before this beat's
                    # score matmuls (write-before-read emission order)
                    for cost, fn in (must or {}).get(i, ()):
                        fn()
                    if i < 16:
                        pss = psS.tile([128, 1024], F32, tag="sc", name="pss")
                        pss_h[i] = pss
                        ch_j = b * 4 + i // 4
                        off_j = (i % 4) * 128
                        for hh, b0 in ((0, 0), (1, 64)):
                            nc.tensor.matmul(
                                pss[:, hh * 512:(hh + 1) * 512],
                                kT_sb[b0:b0 + 64, ch_j, off_j:off_j + 128],
                                qT_sb[b0:b0 + 64, ch_i, :],
                                start=True, stop=True,
                                tile_position=(b0, 0))
                    if i >= 1:
                        pT = ptp.tile([128, 1024], BF16, tag="pT", name="pT")
                        pT_h[i - 1] = pT
                        bias = (mb_sb[:, b * 16 + i - 1:b * 16 + i]
                                if use_mask else 0.0)
                        nc.scalar.activation(
                            pT, pss_h.pop(i - 1),
                            mybir.ActivationFunctionType.Exp,
                            bias=bias, scale=0.125)
                    for _ in range(2):
                        if pre:
                            pre.popleft()()
                    if pos is None:
                        # allocated after the pre-pulls so the previous
                        # chunk's ocp copies have released the pos slots
                        pos = [psP.tile([65, 512], F32, tag="pos",
                                        name=f"pos{_h}") for _h in range(2)]
                    left = budget
                    first = True
                    while fillers and (first or fillers[0][0] <= left):
                        cost, fn = fillers.popleft()
                        left -= max(cost, 220)
                        first = False
                        fn()
                        if left <= 0:
                            break
                    if i >= 1:
                        jb = i - 1
                        for hh, v_sb in ((0, vA_sb), (1, vB_sb)):
                            nc.tensor.matmul(
                                pos[hh],
                                v_sb[:, b * 16 + jb, :],
                                pT_h[jb][:, hh * 512:(hh + 1) * 512],
                                start=(jb == 0), stop=(jb == 15))
                return pos

            def chunk_state(ch):
                st = {}
                return st, gen_load(ch, st)

            # ---- schedule ----
            # all load DMAs for batches 0 and 1 are issued first (they
            # stream while the warm-up projections run); only chunk 0 is
            # projected before the beats start -- chunks 1-3 are pinned
            # (`must`) into batch 0's first query-chunk so their k/v
            # results are emitted just ahead of the beats that read them
            sts = {}
            for ch in range(4):
                st, lg = chunk_state(ch)
                sts[ch] = st
                for _, fn in lg:
                    fn()
            b1sts = {}
            for c in range(4, 8):
                b1sts[c], lg = chunk_state(c)
                for _, fn in lg:
                    fn()
            for ch in range(1):
                for _, fn in gen_qk(ch, sts[ch], wq_sb, qT_sb):
                    fn()
                for _, fn in gen_qk(ch, sts[ch], wk_sb, kT_sb):
                    fn()
                for _, fn in gen_v(ch, sts[ch]):
                    fn()

            def sched(lst, beats):
                """Spread thunks of lst over the given beats, in order."""
                out = {}
                bi = list(beats)
                for idx, t in enumerate(lst):
                    be = bi[min(idx, len(bi) - 1)]
                    out.setdefault(be, []).append(t)
                return out

            def merge(*ds):
                out = {}
                for d in ds:
                    for k, v in d.items():
                        out.setdefault(k, []).extend(v)
                return out

            must0 = merge(
                sched(list(gen_qk(1, sts[1], wk_sb, kT_sb)), (1, 2, 3)),
                sched(list(gen_v(1, sts[1])), (2, 3, 4, 5)),
                sched(list(gen_qk(2, sts[2], wk_sb, kT_sb)), (5, 6, 7)),
                sched(list(gen_v(2, sts[2])), (6, 7, 8, 9)),
                sched(list(gen_qk(3, sts[3], wk_sb, kT_sb)), (9, 10, 11)),
                sched(list(gen_v(3, sts[3])), (10, 11, 12, 13)),
                sched(list(gen_qk(1, sts[1], wq_sb, qT_sb)), (13, 14, 15)),
            )
            must1 = sched(list(gen_qk(2, sts[2], wq_sb, qT_sb)), (4, 5, 6))
            must2 = sched(list(gen_qk(3, sts[3], wq_sb, qT_sb)), (4, 5, 6))
            musts = {0: must0, 1: must1, 2: must2}

            fillers = deque()
            pre = deque()
            for b in range(B):
                if b + 1 < B:
                    if b == 0:
                        nsts = b1sts
                    else:
                        nsts = {}
                        for i in range(4):
                            c = (b + 1) * 4 + i
                            nsts[c], lg = chunk_state(c)
                            fillers.extend(lg)
                if b > 0:
                    for cck in range(KT):
                        fillers.extend(gen_out(b - 1, cck))
                if b + 1 < B:
                    for i in range(4):
                        c = (b + 1) * 4 + i
                        fillers.extend(gen_qk(c, nsts[c], wq_sb, qT_sb))
                        fillers.extend(gen_qk(c, nsts[c], wk_sb, kT_sb))
                        fillers.extend(gen_v(c, nsts[c]))
                for qc in range(4):
                    pos = phase2_qc(b, qc, fillers, pre, budget=700)
                    pre = deque(gen_finalize(b, qc, pos))
                while fillers:
                    fillers.popleft()[1]()
            # tail: last finalize + remaining output rows
            while pre:
                pre.popleft()()
            tailf = deque()
            for cck in range(KT):
                tailf.extend(gen_out(B - 1, cck))
            while tailf:
                tailf.popleft()[1]()

    return nc


def _host_prep(x, key_padding_mask, dt, wq, wk, wv, wo):
    """Shared + per-core input arrays (all numpy)."""
    xT = np.ascontiguousarray(x.reshape(T, E).T).astype(ml_dtypes.bfloat16)

    # RoPE trig tables, rows [c;c;c;c] and [s;-s;s;-s] over 32-row blocks
    inv_freq = (1.0 / (THETA ** (np.arange(0, D, 2, dtype=np.float32) / D)))
    ang = dt.reshape(T).astype(np.float32)[None, :] * inv_freq[:, None]  # [32, T]
    cos = np.cos(ang).astype(np.float32)
    sin = np.sin(ang).astype(np.float32)
    cc = np.concatenate([cos, cos, cos, cos], axis=0).astype(np.float16)
    ssm = np.concatenate([sin, -sin, sin, -sin], axis=0).astype(np.float16)

    use_mask = bool(key_padding_mask.any())
    mb = None
    if use_mask:
        bias = np.where(key_padding_mask.reshape(T), NEG_INF, 0.0).astype(np.float32)
        # [128 j-in-block, B*16 block index]
        mb = np.ascontiguousarray(bias.reshape(B * 16, 128).T)

    # per-head channel permutation: [2r] then [2r+1] -> [r | 32+r]
    perm1 = np.concatenate([np.arange(0, D, 2), np.arange(1, D, 2)])

    per_core = []
    for c in range(NCORES):
        rows = []
        for h in range(c * HPC, (c + 1) * HPC):
            rows.append(h * D + perm1)
        rows = np.concatenate(rows)                      # permuted q/k rows
        rows_v = np.arange(c * CPC, (c + 1) * CPC)       # natural v rows
        # note: the 1/sqrt(D)=0.125 score scale is applied as the exp
        # activation's scale argument on device, not here
        wqT = np.ascontiguousarray(wq[rows].T).astype(ml_dtypes.bfloat16)
        wkT = np.ascontiguousarray(wk[rows].T).astype(ml_dtypes.bfloat16)
        wvT = np.ascontiguousarray(wv[rows_v].T).astype(ml_dtypes.bfloat16)
        woT = np.ascontiguousarray(wo[:, rows_v].T).astype(ml_dtypes.bfloat16)
        m = {"xT": xT, "cc": cc, "ss": ssm,
             "wqT": wqT, "wkT": wkT, "wvT": wvT, "woT": woT}
        if use_mask:
            m["mb"] = mb
        per_core.append(m)
    return per_core, use_mask


def kernel(x, key_padding_mask, dt, wq, wk, wv, wo, bo, _return_results=False):
    x = np.asarray(x, dtype=np.float32)
    key_padding_mask = np.asarray(key_padding_mask)
    dt = np.asarray(dt, dtype=np.float32)
    wq = np.asarray(wq, dtype=np.float32)
    wk = np.asarray(wk, dtype=np.float32)
    wv = np.asarray(wv, dtype=np.float32)
    wo = np.asarray(wo, dtype=np.float32)
    bo = np.asarray(bo, dtype=np.float32)

    in_maps, use_mask = _host_prep(x, key_padding_mask, dt, wq, wk, wv, wo)

    key = use_mask
    if key not in _prog_cache:
        prog = _build_program(use_mask)
        prog.finalize()
        _prog_cache[key] = prog
    nc = _prog_cache[key]

    res = run_bass_kernel_spmd(nc, in_maps, list(range(NCORES)))

    y = np.zeros((E, T), dtype=np.float32)
    for r in res.results:
        y += r["yT"].astype(np.float32)
    out = (y.T + bo[None, :]).reshape(B, S, E).astype(np.float32)
    if _return_results:
        return out, res
    return out
